# revision 1
# baseline (speedup 1.0000x reference)
"""Additive attention (Bahdanau) fused Trainium2 kernel, data-parallel over batch.

Math: with q = Q @ Wq.T + bq, k = K @ Wk.T + bk,
  scores[b,i,j] = tanh( w_s . (q[b,i] + k[b,j]) + b_s )
                = tanh( qs[b,i] + ks[b,j] + cbs )
where qs = Q @ u, ks = K @ v, u = Wq.T @ w_s, v = Wk.T @ w_s,
cbs = (bq+bk).w_s + b_s. The (B,Lq,Lk,H) intermediate is never materialized.
u, v, cbs are tiny weight-only folds (O(H*F)) done host-side and shipped as
extra bf16 columns of the packed tensors; all O(L*F) math runs on device.
tanh bounds scores in [-1,1], so the softmax needs no max-subtraction; masking
is a per-key -120 additive bias on the exp input (host-built from valid_lens;
exp(-120±1) underflows to 0, matching the reference's -1e6 fill). The softmax
denominator comes from a ones-column appended to V inside the attn @ V matmul.

Layout per core (batch element b), all feature-major so projections run on PE:
  qp [128f, h*512+j | u0 u1]             = Q.T + u columns
  kp [128f, c*256+h*128+j | v0 v1 cbs eb0..3] = K.T chunk-tiled + v/cbs/mask
  vp [128k, c*257+g]                     = V chunk rows + ones column
scores kept TRANSPOSED [k, q]: tanh(qs_bc + ks_col bias) then exp(+mask bias)
on ACT — 8 serial [128,512] passes are the critical chain; attn.T @ [V|1] on
the PE trails each exp. All HBM traffic bf16; output host-upcast.

Timing notes (measured): a DMA's completion semaphore fires ~1.5-2.5us after
its descriptor-gen ends (HBM read receipt under load), so the first compute
lands ~4us into the body regardless — the PE spends that window on warm-up
matmuls to open the HAM clock gate (cold PE = 1.2 GHz; needs ~4us sustained
activity to reach 2.4). kp streams first (ks columns), qp second, vp on the
scalar queue behind the ACT-table warm-up; gpsimd untouched (slow SWDGE).

Sharding: batch B=8 across 8 NeuronCores, one batch element per core.
"""

from contextlib import ExitStack

import numpy as np
import ml_dtypes

import concourse.tile as tile
from concourse import bacc, mybir
from concourse.bass import ts
from concourse.bass_utils import run_bass_kernel_spmd

B, LQ, LK = 8, 512, 512
F = 256          # feature dim of Q/K/V
H = 128          # hidden dim of the additive-attention MLP
P = 128          # SBUF partitions
QT = LQ // P     # query chunks per core
KT = LK // P     # key chunks per core
FH = F // P      # feature halves
NCORES = 8
MASK_BIAS = -120.0  # exp(-120 + [-1,1]) == 0.0 in bf16
N_WARM = 7       # PE warm-up matmuls (512 cols each) to open the HAM gate

F32 = mybir.dt.float32
BF16 = mybir.dt.bfloat16
BF = ml_dtypes.bfloat16
FP8 = ml_dtypes.float8_e4m3
F8 = mybir.dt.float8e4
QSCALE = 64.0    # u columns shipped as u*QSCALE in fp8; tanh rescales

# qp extra columns
U0 = FH * LQ                 # 1024: u half 0, 1025: u half 1
QP_W = U0 + FH
# kpa holds key chunks 0-1 plus all the small folds; kpb holds chunks 2-3
V0 = 2 * F                   # 512: v half 0, 513: v half 1
CBS_C = V0 + FH              # 514: cbs replicated
EB0 = CBS_C + 1              # 515..518: exp mask bias per key chunk
KPA_W = EB0 + KT
KPB_W = 2 * F

TRACE = False
LAST_RESULT = None


def _emit(tc, d):
    nc = tc.nc
    X = mybir.AxisListType
    A = mybir.AluOpType
    AF = mybir.ActivationFunctionType

    with ExitStack() as ctx:
        consts = ctx.enter_context(tc.tile_pool(name="consts", bufs=1))
        big = ctx.enter_context(tc.tile_pool(name="big", bufs=1))
        st_pool = ctx.enter_context(tc.tile_pool(name="st", bufs=2, space="PSUM"))
        et_pool = ctx.enter_context(tc.tile_pool(name="et", bufs=3))
        ps_qs = ctx.enter_context(tc.tile_pool(name="ps_qs", bufs=1, space="PSUM"))
        ps_ks = ctx.enter_context(tc.tile_pool(name="ps_ks", bufs=1, space="PSUM"))
        ps_acc = ctx.enter_context(tc.tile_pool(name="ps_acc", bufs=1, space="PSUM"))

        # ---- DMA issue: ONE queue, strict FIFO in need order (a second
        # queue's transfer would round-robin into the stream at the SDMA
        # engines and delay the bytes the chain is waiting on).
        kpa = big.tile([P, KPA_W], BF16)
        nc.sync.dma_start(kpa, d["kpa"])
        qp = big.tile([P, QP_W], F8)
        nc.sync.dma_start(qp, d["qp"])
        kpb = big.tile([P, KPB_W], BF16)
        nc.sync.dma_start(kpb, d["kpb"])
        vp = big.tile([P, KT * (F + 1)], BF16)
        nc.sync.dma_start(vp, d["vp"])

        # PE warm-up operand first so the PE starts as early as possible
        junk = consts.tile([P, LQ], BF16)
        nc.vector.memset(junk, 0.0)

        # ACT table prefetch: dummy Exp loads exp_and_others (also holds tanh
        # and copy) while the DMAs stream; a dummy Tanh warms that path too.
        warm = consts.tile([1, 1], F32)
        nc.vector.memset(warm, 0.0)
        nc.scalar.activation(warm, warm, AF.Exp)
        nc.scalar.activation(warm, warm, AF.Tanh)

        # ---- PE warm-up on the memset tile while the input DMAs land: the
        # HAM clock gate needs ~4us of sustained matmul activity to open.
        qs_bc = ps_qs.tile([P, LQ], F32)  # warm-up target; later the qs row
        for _ in range(N_WARM):
            nc.tensor.matmul(qs_bc, junk[:, 0:P], junk, start=True, stop=True)

        # ---- ks columns: ks[c][i] = sum_f K.T[f, c*128+i] v[f]
        ks_ps = ps_ks.tile([P, KT], F32)

        def ks_mms(cs, kt):
            for c in cs:
                for h in range(FH):
                    nc.tensor.matmul(ks_ps[:, c:c + 1],
                                     kt[:, (c % 2) * 2 * P + h * P:
                                         (c % 2) * 2 * P + (h + 1) * P],
                                     kpa[:, V0 + h:V0 + h + 1],
                                     start=(h == 0), stop=(h == 1))

        ks_mms((0, 1), kpa)

        # ---- qs broadcast row: qs_bc[p, j] = sum_f u[f] Q.T[f, j] for all p
        for h in range(FH):
            nc.tensor.matmul(qs_bc, qp[:, U0 + h:U0 + h + 1].broadcast_to([P, P]),
                             qp[:, ts(h, LQ)], start=(h == 0), stop=(h == 1))

        ks_mms((2, 3), kpb)

        # tanh bias columns: ks + cbs
        cbsf = consts.tile([P, 1], F32)
        nc.vector.tensor_copy(cbsf, kpa[:, CBS_C:CBS_C + 1])
        ksb = [consts.tile([P, 2], F32, name=f"ksb{i}") for i in range(2)]
        for i in range(2):
            nc.vector.tensor_scalar(ksb[i], ks_ps[:, 2 * i:2 * i + 2],
                                    cbsf, None, A.add)

        # ---- scores.T -> exp (bf16) -> attn.T @ [V | 1] ----
        accs = [ps_acc.tile([P, F + 1], F32, tag=f"acc{qc}", name=f"acc{qc}")
                for qc in range(QT)]
        for c in range(KT):
            sT = st_pool.tile([P, LQ], F32, tag="sT")
            nc.scalar.activation(sT, qs_bc, AF.Tanh, scale=1.0 / QSCALE,
                                 bias=ksb[c // 2][:, c % 2:c % 2 + 1])
            eT = et_pool.tile([P, LQ], BF16, tag="eT")
            nc.scalar.activation(eT, sT, AF.Exp, bias=kpa[:, EB0 + c:EB0 + c + 1])
            vtile = vp[:, c * (F + 1):(c + 1) * (F + 1)]
            for qc in range(QT):
                nc.tensor.matmul(accs[qc], eT[:, ts(qc, P)], vtile,
                                 start=(c == 0), stop=(c == KT - 1))

        # ---- normalize (ACT takes qc 0/1, DVE takes qc 2/3) into ONE out
        # tile, stored with a single DMA (two half-stores trickle worse) ----
        ob = big.tile([P, QT * F], BF16)
        recs = [consts.tile([P, 1], F32, tag=f"rec{qc}", name=f"rec{qc}")
                for qc in range(QT)]
        nc.vector.reciprocal(recs[0], accs[0][:, F:F + 1])
        nc.vector.reciprocal(recs[2], accs[2][:, F:F + 1])
        nc.scalar.activation(ob[:, 0:F], accs[0][:, 0:F], AF.Copy,
                             bias=0.0, scale=recs[0])
        nc.vector.tensor_scalar(ob[:, 2 * F:3 * F], accs[2][:, 0:F], recs[2], None, A.mult)
        nc.vector.reciprocal(recs[1], accs[1][:, F:F + 1])
        nc.vector.reciprocal(recs[3], accs[3][:, F:F + 1])
        nc.scalar.activation(ob[:, F:2 * F], accs[1][:, 0:F], AF.Copy,
                             bias=0.0, scale=recs[1])
        nc.vector.tensor_scalar(ob[:, 3 * F:4 * F], accs[3][:, 0:F], recs[3], None, A.mult)
        nc.sync.dma_start(d["out"], ob)

        # late read of the warm-up/qs psum bank so warm-up matmuls stay live
        warm_junk = consts.tile([P, 1], F32)
        nc.vector.reduce_sum(warm_junk, qs_bc[:, 0:8], axis=X.X)


_NC = None


def _build_nc():
    nc = bacc.Bacc("TRN2", target_bir_lowering=False, debug=False, num_devices=1)
    d = {}
    d["qp"] = nc.dram_tensor("qp", [P, QP_W], F8, kind="ExternalInput").ap()
    d["kpa"] = nc.dram_tensor("kpa", [P, KPA_W], BF16, kind="ExternalInput").ap()
    d["kpb"] = nc.dram_tensor("kpb", [P, KPB_W], BF16, kind="ExternalInput").ap()
    d["vp"] = nc.dram_tensor("vp", [P, KT * (F + 1)], BF16, kind="ExternalInput").ap()
    d["out"] = nc.dram_tensor("out", [P, QT * F], BF16, kind="ExternalOutput").ap()

    with tile.TileContext(nc) as tc:
        _emit(tc, d)
    nc.compile()
    return nc


def get_nc():
    global _NC
    if _NC is None:
        _NC = _build_nc()
    return _NC


def make_in_maps(queries, keys, values, valid_lens, Wq, bq, Wk, bk, w_s, b_s):
    f32 = lambda a: np.asarray(a, dtype=np.float32)
    qs, ks, vs = f32(queries), f32(keys), f32(values)
    vl = np.asarray(valid_lens)
    ws = f32(w_s)
    u = f32(Wq).T @ ws            # [F]
    v = f32(Wk).T @ ws            # [F]
    cbs = float(ws @ (f32(bq) + f32(bk)) + f32(b_s).reshape(-1)[0])
    iota = np.arange(P)[:, None] + P * np.arange(KT)[None, :]  # [P, KT]

    in_maps = []
    for b in range(NCORES):
        # qp[p, h*512 + j] = Q[j, h*128 + p]; then u columns
        qpk = np.empty((P, QP_W), np.float32)
        qpk[:, 0:FH * LQ] = qs[b].T.reshape(FH, P, LQ).transpose(1, 0, 2).reshape(P, FH * LQ)
        qpk[:, U0:U0 + FH] = QSCALE * u.reshape(FH, P).T
        # k chunk-pair tiles: [p, c*256 + h*128 + j] = K[c*128 + j, h*128 + p]
        kd = ks[b].T.reshape(FH, P, KT, P).transpose(1, 2, 0, 3).reshape(P, KT * F)
        kpk = np.empty((P, KPA_W), np.float32)
        kpk[:, 0:2 * F] = kd[:, 0:2 * F]
        kpk[:, V0:V0 + FH] = v.reshape(FH, P).T
        kpk[:, CBS_C] = cbs
        kpk[:, EB0:EB0 + KT] = np.where(iota >= int(vl[b]), MASK_BIAS, 0.0)
        # vp[p, c*257 + g] = V[c*128 + p, g] (g<256) | 1.0 (g=256)
        va = np.ones((KT, P, F + 1), np.float32)
        va[:, :, :F] = vs[b].reshape(KT, P, F)
        in_maps.append({
            "qp": qpk.astype(FP8),
            "kpa": kpk.astype(BF),
            "kpb": np.ascontiguousarray(kd[:, 2 * F:4 * F]).astype(BF),
            "vp": np.ascontiguousarray(
                va.transpose(1, 0, 2).reshape(P, KT * (F + 1))).astype(BF),
        })
    return in_maps


def kernel(queries, keys, values, valid_lens, Wq, bq, Wk, bk, w_s, b_s):
    global LAST_RESULT
    nc = get_nc()
    in_maps = make_in_maps(queries, keys, values, valid_lens, Wq, bq, Wk, bk, w_s, b_s)
    res = run_bass_kernel_spmd(nc, in_maps, list(range(NCORES)), trace=TRACE)
    LAST_RESULT = res
    out = np.stack([np.asarray(res.results[b]["out"]) for b in range(NCORES)], axis=0)
    # [P, QT*F] (bf16) -> [LQ, F] fp32
    out = out.astype(np.float32).reshape(B, P, QT, F).transpose(0, 2, 1, 3).reshape(B, LQ, F)
    return np.ascontiguousarray(out)



# revision 3
# speedup vs baseline: 1.1130x; 1.1130x over previous
"""Additive attention (Bahdanau) fused Trainium2 kernel, data-parallel over batch.

Math: with q = Q @ Wq.T + bq, k = K @ Wk.T + bk,
  scores[b,i,j] = tanh( w_s . (q[b,i] + k[b,j]) + b_s )
                = tanh( qs[b,i] + ks[b,j] + cbs )
where qs = Q @ u, ks = K @ v, u = Wq.T @ w_s, v = Wk.T @ w_s,
cbs = (bq+bk).w_s + b_s. The (B,Lq,Lk,H) intermediate is never materialized.
u, v, cbs are tiny weight-only folds (O(H*F)) done host-side and shipped as
extra bf16 columns of the packed tensors; all O(L*F) math runs on device.

Softmax weights: exp(tanh(x)) is itself a bounded sigmoid-shaped function;
we use the minimax fit  exp(tanh(x)) ~= SIGB * (sigmoid(SIGC*x + SIGD) + AOB)
(max rel err 3.1e-3 on |x|<=5, saturates to the right asymptotes outside).
The global factor SIGB cancels in the softmax, so ONE ACT pass per key chunk
(Sigmoid; the affine pre-map rides the instruction's free scale/bias) plus a
2x-rate bf16 DVE add of AOB replaces the old tanh+exp pair -- the ACT chain
halves from 8 to 4 passes.  Masking is host-side zeroing of the [V | 1] rows
for keys j >= valid_len (exact: removes them from numerator AND denominator,
matching the reference's -1e6 fill), so no mask bias is needed on device.
The softmax denominator comes from a ones-column appended to V inside the
attn @ V matmul.

Layout per core (batch element b), all feature-major so projections run on PE:
  qp  [128f, h*512+j | u0 u1]          = Q.T + u columns (fp8, u pre-scaled)
  kpa [128f, c*256+h*128+j | v0 v1 cbs2] = K.T chunks 0-1 + v / (cbs + d/c)
  kpb [128f, c*256+h*128+j]            = K.T chunks 2-3
  vp0/vp1 [128k, cc*257+g]             = [V|1] rows (chunks 0-1 / 2-3),
                                         masked rows zeroed
scores stay TRANSPOSED [k, q]: eT_c = Sigmoid(SIGC/QSCALE * qs_bc + ksb_c)
on ACT, eS_c = eT_c + AOB on DVE (bf16 2x), then eS.T @ [V|1] on the PE
trails each chunk; per-qc reciprocal+scale, two column-sliced stores.

Timing notes (measured): a DMA's completion semaphore fires ~1.5-2.5us after
its descriptor-gen ends (HBM read receipt under load), so the first compute
lands ~4us into the body regardless -- the PE spends that window on warm-up
matmuls to open the HAM clock gate (cold PE = 1.2 GHz; needs ~4us sustained
activity to reach 2.4). kpa streams first (ks columns), qp second, kpb, then
the two vp halves; all on ONE queue, strict FIFO in need order.

Sharding: batch B=8 across 8 NeuronCores, one batch element per core.
"""

from contextlib import ExitStack

import numpy as np
import ml_dtypes

import concourse.tile as tile
from concourse import bacc, mybir
from concourse.bass import ts
from concourse.bass_utils import run_bass_kernel_spmd

B, LQ, LK = 8, 512, 512
F = 256          # feature dim of Q/K/V
H = 128          # hidden dim of the additive-attention MLP
P = 128          # SBUF partitions
QT = LQ // P     # query chunks per core
KT = LK // P     # key chunks per core
FH = F // P      # feature halves
NCORES = 8
N_WARM = 7       # PE warm-up matmuls (512 cols each) to open the HAM gate

F32 = mybir.dt.float32
BF16 = mybir.dt.bfloat16
BF = ml_dtypes.bfloat16
FP8 = ml_dtypes.float8_e4m3
F8 = mybir.dt.float8e4
QSCALE = 64.0    # u columns shipped as u*QSCALE in fp8; ACT scale rescales

# exp(tanh(x)) ~= SIGB*(sigmoid(SIGC*x+SIGD) + AOB); SIGB cancels in softmax
SIGA = 0.3690355303146853
SIGB = 2.3407045472544117
SIGC = 2.142469687764282
SIGD = -0.9968575347084756
AOB = SIGA / SIGB            # 0.15766 additive shift on the sigmoid output

# qp extra columns
U0 = FH * LQ                 # 1024: u half 0, 1025: u half 1
QP_W = U0 + FH
# kpa holds key chunks 0-1 plus the small folds; kpb holds chunks 2-3
V0 = 2 * F                   # 512: v half 0, 513: v half 1
CBS_C = V0 + FH              # 514: cbs + SIGD/SIGC replicated
KPA_W = CBS_C + 1
KPB_W = 2 * F
VP_W = 2 * (F + 1)           # two [V|1] chunks per vp tensor

TRACE = False
LAST_RESULT = None


def _emit(tc, d):
    nc = tc.nc
    X = mybir.AxisListType
    A = mybir.AluOpType
    AF = mybir.ActivationFunctionType

    with ExitStack() as ctx:
        consts = ctx.enter_context(tc.tile_pool(name="consts", bufs=1))
        big = ctx.enter_context(tc.tile_pool(name="big", bufs=1))
        es_pool = ctx.enter_context(tc.tile_pool(name="es", bufs=2))
        et_pool = ctx.enter_context(tc.tile_pool(name="et", bufs=2))
        ps_qs = ctx.enter_context(tc.tile_pool(name="ps_qs", bufs=1, space="PSUM"))
        ps_ks = ctx.enter_context(tc.tile_pool(name="ps_ks", bufs=1, space="PSUM"))
        ps_acc = ctx.enter_context(tc.tile_pool(name="ps_acc", bufs=1, space="PSUM"))

        # ---- DMA issue: ONE queue, strict FIFO in need order (a second
        # queue's transfer would round-robin into the stream at the SDMA
        # engines and delay the bytes the chain is waiting on).
        kpa = big.tile([P, KPA_W], BF16)
        nc.sync.dma_start(kpa, d["kpa"])
        qp = big.tile([P, QP_W], F8)
        nc.sync.dma_start(qp, d["qp"])
        kpb = big.tile([P, KPB_W], BF16)
        nc.sync.dma_start(kpb, d["kpb"])
        vp0 = big.tile([P, VP_W], BF16)
        nc.sync.dma_start(vp0, d["vp0"])
        vp1 = big.tile([P, VP_W], BF16)
        nc.sync.dma_start(vp1, d["vp1"])

        # PE warm-up operand first so the PE starts as early as possible
        junk = consts.tile([P, LQ], BF16)
        nc.vector.memset(junk, 0.0)

        # ACT table prefetch: dummy Sigmoid loads sigmoid_and_others (also
        # holds copy) while the DMAs stream.
        warm = consts.tile([1, 1], F32)
        nc.vector.memset(warm, 0.0)
        nc.scalar.activation(warm, warm, AF.Sigmoid)

        # ---- PE warm-up on the memset tile while the input DMAs land: the
        # HAM clock gate needs ~4us of sustained matmul activity to open.
        qs_bc = ps_qs.tile([P, LQ], F32)  # warm-up target; later the qs row
        for _ in range(N_WARM):
            nc.tensor.matmul(qs_bc, junk[:, 0:P], junk, start=True, stop=True)

        # ---- ks columns: ks[c][i] = sum_f K.T[f, c*128+i] v[f]
        ks01 = ps_ks.tile([P, 2], F32, name="ks01")
        ks23 = ps_ks.tile([P, 2], F32, name="ks23")

        def ks_mms(dst, cs, kt):
            for c in cs:
                for h in range(FH):
                    nc.tensor.matmul(dst[:, c % 2:c % 2 + 1],
                                     kt[:, (c % 2) * 2 * P + h * P:
                                         (c % 2) * 2 * P + (h + 1) * P],
                                     kpa[:, V0 + h:V0 + h + 1],
                                     start=(h == 0), stop=(h == 1))

        ks_mms(ks01, (0, 1), kpa)

        # sigmoid bias columns: ksb = SIGC*(ks + cbs + SIGD/SIGC)
        cbsf = consts.tile([P, 1], F32)
        nc.vector.tensor_copy(cbsf, kpa[:, CBS_C:CBS_C + 1])
        ksb = [consts.tile([P, 2], F32, name=f"ksb{i}") for i in range(2)]
        nc.vector.tensor_scalar(ksb[0], ks01, cbsf, SIGC, A.add, A.mult)

        # ---- qs broadcast row: qs_bc[p, j] = sum_f u[f] Q.T[f, j] for all p
        for h in range(FH):
            nc.tensor.matmul(qs_bc, qp[:, U0 + h:U0 + h + 1].broadcast_to([P, P]),
                             qp[:, ts(h, LQ)], start=(h == 0), stop=(h == 1))

        ks_mms(ks23, (2, 3), kpb)
        nc.vector.tensor_scalar(ksb[1], ks23, cbsf, SIGC, A.add, A.mult)

        # ---- fused score->weight: eT = sigmoid(SIGC*(qs+ks+cbs)+SIGD) on
        # ACT (ONE pass per chunk), then eS = eT + AOB on DVE (bf16 2x rate);
        # eS.T @ [V | 1] on the PE trails each chunk.
        accs = [ps_acc.tile([P, F + 1], F32, tag=f"acc{qc}", name=f"acc{qc}")
                for qc in range(QT)]
        vtiles = [vp0[:, 0:F + 1], vp0[:, F + 1:2 * (F + 1)],
                  vp1[:, 0:F + 1], vp1[:, F + 1:2 * (F + 1)]]
        for c in range(KT):
            eT = et_pool.tile([P, LQ], BF16, tag="eT")
            nc.scalar.activation(eT, qs_bc, AF.Sigmoid, scale=SIGC / QSCALE,
                                 bias=ksb[c // 2][:, c % 2:c % 2 + 1])
            eS = es_pool.tile([P, LQ], BF16, tag="eS")
            nc.vector.tensor_scalar(eS, eT, AOB, None, A.add)
            for qc in range(QT):
                nc.tensor.matmul(accs[qc], eS[:, ts(qc, P)], vtiles[c],
                                 start=(c == 0), stop=(c == KT - 1))

        # ---- normalize (ACT takes qc 0/1, DVE takes qc 2/3) into ONE out
        # tile, stored with two column-sliced DMAs so the first half streams
        # while the second half finishes ----
        ob01 = big.tile([P, 2 * F], BF16)
        ob23 = big.tile([P, 2 * F], BF16)
        recs = [consts.tile([P, 1], F32, tag=f"rec{qc}", name=f"rec{qc}")
                for qc in range(QT)]
        nc.vector.reciprocal(recs[0], accs[0][:, F:F + 1])
        nc.vector.reciprocal(recs[1], accs[1][:, F:F + 1])
        nc.scalar.activation(ob01[:, 0:F], accs[0][:, 0:F], AF.Copy,
                             bias=0.0, scale=recs[0])
        nc.scalar.activation(ob01[:, F:2 * F], accs[1][:, 0:F], AF.Copy,
                             bias=0.0, scale=recs[1])
        nc.sync.dma_start(d["out"][:, 0:2 * F], ob01)
        nc.vector.reciprocal(recs[2], accs[2][:, F:F + 1])
        nc.vector.reciprocal(recs[3], accs[3][:, F:F + 1])
        nc.vector.tensor_scalar(ob23[:, 0:F], accs[2][:, 0:F], recs[2], None, A.mult)
        nc.vector.tensor_scalar(ob23[:, F:2 * F], accs[3][:, 0:F], recs[3], None, A.mult)
        nc.sync.dma_start(d["out"][:, 2 * F:4 * F], ob23)

        # late read of the warm-up/qs psum bank so warm-up matmuls stay live
        warm_junk = consts.tile([P, 1], F32)
        nc.vector.reduce_sum(warm_junk, qs_bc[:, 0:8], axis=X.X)


_NC = None


def _build_nc():
    nc = bacc.Bacc("TRN2", target_bir_lowering=False, debug=False, num_devices=1)
    d = {}
    d["qp"] = nc.dram_tensor("qp", [P, QP_W], F8, kind="ExternalInput").ap()
    d["kpa"] = nc.dram_tensor("kpa", [P, KPA_W], BF16, kind="ExternalInput").ap()
    d["kpb"] = nc.dram_tensor("kpb", [P, KPB_W], BF16, kind="ExternalInput").ap()
    d["vp0"] = nc.dram_tensor("vp0", [P, VP_W], BF16, kind="ExternalInput").ap()
    d["vp1"] = nc.dram_tensor("vp1", [P, VP_W], BF16, kind="ExternalInput").ap()
    d["out"] = nc.dram_tensor("out", [P, QT * F], BF16, kind="ExternalOutput").ap()

    with tile.TileContext(nc) as tc:
        _emit(tc, d)
    nc.compile()
    return nc


def get_nc():
    global _NC
    if _NC is None:
        _NC = _build_nc()
    return _NC


def make_in_maps(queries, keys, values, valid_lens, Wq, bq, Wk, bk, w_s, b_s):
    f32 = lambda a: np.asarray(a, dtype=np.float32)
    qs, ks, vs = f32(queries), f32(keys), f32(values)
    vl = np.asarray(valid_lens)
    ws = f32(w_s)
    u = f32(Wq).T @ ws            # [F]
    v = f32(Wk).T @ ws            # [F]
    cbs = float(ws @ (f32(bq) + f32(bk)) + f32(b_s).reshape(-1)[0])
    cbs2 = cbs + SIGD / SIGC      # folded so ksb = SIGC*(ks + cbs2)

    in_maps = []
    for b in range(NCORES):
        # qp[p, h*512 + j] = Q[j, h*128 + p]; then u columns
        qpk = np.empty((P, QP_W), np.float32)
        qpk[:, 0:FH * LQ] = qs[b].T.reshape(FH, P, LQ).transpose(1, 0, 2).reshape(P, FH * LQ)
        qpk[:, U0:U0 + FH] = QSCALE * u.reshape(FH, P).T
        # k chunk-pair tiles: [p, c*256 + h*128 + j] = K[c*128 + j, h*128 + p]
        kd = ks[b].T.reshape(FH, P, KT, P).transpose(1, 2, 0, 3).reshape(P, KT * F)
        kpk = np.empty((P, KPA_W), np.float32)
        kpk[:, 0:2 * F] = kd[:, 0:2 * F]
        kpk[:, V0:V0 + FH] = v.reshape(FH, P).T
        kpk[:, CBS_C] = cbs2
        # vp[p, cc*257 + g] = V[c*128 + p, g] (g<256) | 1.0 (g=256),
        # rows for masked keys (c*128+p >= valid_len) zeroed
        va = np.ones((KT, P, F + 1), np.float32)
        va[:, :, :F] = vs[b].reshape(KT, P, F)
        kidx = (np.arange(KT)[:, None] * P + np.arange(P)[None, :])
        va[kidx >= int(vl[b])] = 0.0
        vap = va.transpose(1, 0, 2).reshape(P, KT * (F + 1))
        in_maps.append({
            "qp": qpk.astype(FP8),
            "kpa": kpk.astype(BF),
            "kpb": np.ascontiguousarray(kd[:, 2 * F:4 * F]).astype(BF),
            "vp0": np.ascontiguousarray(vap[:, 0:VP_W]).astype(BF),
            "vp1": np.ascontiguousarray(vap[:, VP_W:2 * VP_W]).astype(BF),
        })
    return in_maps


def kernel(queries, keys, values, valid_lens, Wq, bq, Wk, bk, w_s, b_s):
    global LAST_RESULT
    nc = get_nc()
    in_maps = make_in_maps(queries, keys, values, valid_lens, Wq, bq, Wk, bk, w_s, b_s)
    res = run_bass_kernel_spmd(nc, in_maps, list(range(NCORES)), trace=TRACE)
    LAST_RESULT = res
    out = np.stack([np.asarray(res.results[b]["out"]) for b in range(NCORES)], axis=0)
    # [P, QT*F] (bf16) -> [LQ, F] fp32
    out = out.astype(np.float32).reshape(B, P, QT, F).transpose(0, 2, 1, 3).reshape(B, LQ, F)
    return np.ascontiguousarray(out)
